# revision 2
# baseline (speedup 1.0000x reference)
"""Trainium2 Bass kernel for nn_BaseAttention (causal MHA, b=2, n=2048, d=1024, 16 heads).

Sharding (8 cores): core c handles batch c//4 and heads 4*(c%4)..4*(c%4)+3.
- W_q/W_k/W_v column-sharded (256 cols/core), W_o row-sharded (256 rows/core).
- Each core computes a partial output [2048, 1024] in bf16; host sums the 4
  partials per batch (row-parallel out-projection), adds b_o, stacks batches.

Per-core kernel (bf16 data path, fp32 PSUM accumulation):
  - x is transposed + bf16-cast on the host; weights pre-laid-out on host.
  - startup: c-granular wq/x DMAs + c-outer Q/K round-0 projection so the
    first matmul starts as soon as the first ~200KB lands (cuts DMA head).
  - Q^T/K^T projections emitted transposed; V natural with a ones column per
    head ([V|1] trick: AV yields ctx^T rows 0..63 + softmax row-sum at 64).
  - attention per (head-pair, q-tile j): S^T = K_h @ Q_h^T on PE (even/odd
    heads on disjoint PE row-halves), exp on ACT narrowed to the causal-valid
    region, causal mask via gpsimd affine_select, AV pipelined one i-pair
    behind S, normalization via DVE reciprocal_approx_fast + gpsimd
    partition_broadcast + DVE multiply.
  - projection work of round g+1 and out-projection chunks are woven between
    attention steps (pacing 8/7 so round-boundary qt/kt copies land early);
    out-projection spread over rounds 1-3 to spread PSUM + DMA load.
  - no bias on device: host adds b_o during the partial-sum (free).
  - final out-projection: u=0 halves run on PSUM banks freed by the last exp
    (no wait on the last normalize); the last normalize is chunked so its
    first 128 q-cols are ready ~2us earlier; drains alternate ACT/DVE copies
    (bf16) so the PE never idles and stays at full p-state.
  - all output DMAs on the sync queue (fans out over all 16 DMA engines);
    output in bf16 to halve drain bytes.
"""
import sys, types

sys.path.insert(0, "/opt/trn_rl_repo")


def _install_ntff_shim():
    # antenv.axon_hooks is absent in this image; register the NTFF profile
    # hook via ctypes so run_bass_kernel_spmd(trace=True) works under axon.
    if "antenv.axon_hooks" in sys.modules:
        return
    try:
        sys.path.insert(0, "/root/.axon_site")
        from trn_agent_boot.trn_boot import _ntff_profile_via_ctypes

        hook = _ntff_profile_via_ctypes("/opt/axon/libaxon_pjrt.so")
        mod = types.ModuleType("antenv.axon_hooks")
        mod.get_axon_ntff_profile_hook = lambda: hook
        mod.set_axon_ntff_profile_hook = lambda h: None
        sys.modules["antenv.axon_hooks"] = mod
    except Exception:
        pass


_install_ntff_shim()

import numpy as np
import ml_dtypes
import concourse.bass as bass
import concourse.mybir as mybir
import concourse.tile as tile
from concourse import bacc
from concourse.bass_utils import run_bass_kernel_spmd
from contextlib import ExitStack

f32 = mybir.dt.float32
bf16 = mybir.dt.bfloat16
EXP = mybir.ActivationFunctionType.Exp
COPY = mybir.ActivationFunctionType.Copy

SEQ = 2048          # sequence length
DIN = 1024          # model dim (8 chunks of 128)
QC = 256            # q/k/v cols per core (4 heads x 64)
HD = 64             # head dim
NH = 4              # heads per core
NG = 4              # row groups of 512
VST = NH * 65       # Vones stride per row chunk (4 heads x (64 V + 1 ones))

TRACE = False
LAST_RESULTS = None


def build_nc():
    nc = bacc.Bacc()
    x_d = nc.dram_tensor("x", [DIN, SEQ], bf16, kind="ExternalInput")  # pre-transposed on host
    wq_d = nc.dram_tensor("wq", [128, 8 * QC], bf16, kind="ExternalInput")
    wk_d = nc.dram_tensor("wk", [128, 8 * QC], bf16, kind="ExternalInput")
    wv_d = nc.dram_tensor("wv", [128, 8 * QC], bf16, kind="ExternalInput")
    wo_d = nc.dram_tensor("wo", [128, 2 * DIN], bf16, kind="ExternalInput")
    out_d = nc.dram_tensor("out", [SEQ, DIN], bf16, kind="ExternalOutput")

    with tile.TileContext(nc, pool_alloc_mode="queue") as tc, ExitStack() as ctx:
        wr = ctx.enter_context(tc.tile_pool(name="wr", bufs=1))
        cst = ctx.enter_context(tc.tile_pool(name="cst", bufs=1))
        big = ctx.enter_context(tc.tile_pool(name="big", bufs=1))
        ptp = ctx.enter_context(tc.tile_pool(name="ptp", bufs=8))
        nrm = ctx.enter_context(tc.tile_pool(name="nrm", bufs=3))
        ob = ctx.enter_context(tc.tile_pool(name="ob", bufs=6))
        ps = ctx.enter_context(tc.tile_pool(name="ps", bufs=1, space="PSUM"))

        # ---- DMAs, ordered by first use (weights pre-laid-out on host).
        # weights on the scalar HWDGE queue, x on sync: parallel issue.
        # First pieces tiny so the first Q-proj matmul can start ~9.5us in.
        wq_sb = wr.tile([128, 8 * QC], bf16, name="wq_sb")
        nc.scalar.dma_start(wq_sb[:, 0:QC], wq_d[:, 0:QC])          # c0
        nc.scalar.dma_start(wq_sb[:, QC:4 * QC], wq_d[:, QC:4 * QC])
        nc.scalar.dma_start(wq_sb[:, 4 * QC:], wq_d[:, 4 * QC:])

        # x arrives pre-transposed: straight contiguous copy DMAs at full BW.
        # xT[p, c*2048+r] = x[r, c*128+p] = x_d[c*128+p, r]
        xT = big.tile([128, 8 * SEQ], bf16)
        xview = xT[:].rearrange("p (c r) -> p c r", r=SEQ)
        dview = x_d.rearrange("(c p) r -> p c r", p=128)
        nc.sync.dma_start(xview[:, 0:1, 0:512], dview[:, 0:1, 0:512])
        nc.sync.dma_start(xview[:, 1:2, 0:512], dview[:, 1:2, 0:512])
        nc.sync.dma_start(xview[:, 2:4, 0:512], dview[:, 2:4, 0:512])
        nc.sync.dma_start(xview[:, 4:8, 0:512], dview[:, 4:8, 0:512])

        wk_sb = wr.tile([128, 8 * QC], bf16, name="wk_sb")
        nc.scalar.dma_start(wk_sb[:, 0:4 * QC], wk_d[:, 0:4 * QC])
        nc.scalar.dma_start(wk_sb[:, 4 * QC:], wk_d[:, 4 * QC:])
        wv_sb = wr.tile([128, 8 * QC], bf16, name="wv_sb")
        nc.scalar.dma_start(wv_sb[:], wv_d[:])

        for g in range(1, NG):
            nc.sync.dma_start(
                xview[:, :, g * 512:(g + 1) * 512],
                dview[:, :, g * 512:(g + 1) * 512],
            )
        wo_sb = cst.tile([128, 2 * DIN], bf16)
        nc.scalar.dma_start(wo_sb[:], wo_d[:])

        # ---- persistent activations ----
        qt_sb = [big.tile([128, SEQ], bf16, name=f"qt{t}") for t in range(2)]
        kt_sb = [big.tile([128, SEQ], bf16, name=f"kt{t}") for t in range(2)]
        vones = big.tile([128, 16 * VST], bf16)
        ctxt = [big.tile([128, SEQ], bf16, name=f"ctxt{t}") for t in range(2)]

        vview = vones.rearrange("p (r h e) -> p r h e", h=NH, e=65)
        nc.vector.memset(vview[:, :, :, 64], 1.0)

        # ---- emission helpers ----
        def emit_qk(g, t, wt, dst):
            prj = ps.tile([128, 512], f32, tag="b", bufs=2, name="prj")
            for c in range(8):
                nc.tensor.matmul(
                    prj[:],
                    wt[:, c * QC + t * 128: c * QC + t * 128 + 128],
                    xT[:, c * SEQ + g * 512: c * SEQ + g * 512 + 512],
                    start=(c == 0),
                    stop=(c == 7),
                )
            nc.vector.tensor_copy(dst[t][:, g * 512:(g + 1) * 512], prj[:])

        def emit_v(g, rc):
            rcg = 4 * g + rc
            vps = ps.tile([128, 256], f32, tag="b", bufs=2, name="vps")
            for c in range(8):
                nc.tensor.matmul(
                    vps[:],
                    xT[:, c * SEQ + rcg * 128: c * SEQ + rcg * 128 + 128],
                    wv_sb[:, c * QC:(c + 1) * QC],
                    start=(c == 0),
                    stop=(c == 7),
                )
            nc.vector.tensor_copy(
                vview[:, rcg, :, 0:64],
                vps[:].rearrange("p (h e) -> p h e", e=HD),
            )

        def drain_out(rc, n, src, eng):
            osb = ob.tile([128, 512], bf16, tag="o", name="osb")
            if eng == 0:
                nc.vector.tensor_copy(osb[:], src)
            else:
                nc.scalar.activation(osb[:], src, COPY)
            nc.sync.dma_start(
                out_d[rc * 128:(rc + 1) * 128, n * 512:(n + 1) * 512], osb[:]
            )

        def emit_outproj(rc, n, tag="b"):
            ops = ps.tile([128, 512], f32, tag=tag, bufs=2, name="ops")
            for u in range(2):
                nc.tensor.matmul(
                    ops[:],
                    ctxt[u][:, rc * 128:(rc + 1) * 128],
                    wo_sb[:, u * DIN + n * 512: u * DIN + n * 512 + 512],
                    start=(u == 0),
                    stop=(u == 1),
                )
            drain_out(rc, n, ops[:], (rc + n) % 2)

        def proj_chunks(g):
            for t in range(2):
                yield lambda t=t: emit_qk(g, t, wq_sb, qt_sb)
                yield lambda t=t: emit_qk(g, t, wk_sb, kt_sb)
            for rc in range(4):
                yield lambda rc=rc: emit_v(g, rc)

        # ---- round 0 Q/K projection, c-outer so the first matmul needs only
        # the first wq/x chunks; V(g0) is woven into j=0 as filler.
        qp = [ps.tile([128, 512], f32, tag="b", bufs=2, name=f"qp{t}")
              for t in range(2)]
        for c in range(8):
            for t in range(2):
                nc.tensor.matmul(
                    qp[t][:],
                    wq_sb[:, c * QC + t * 128: c * QC + t * 128 + 128],
                    xT[:, c * SEQ: c * SEQ + 512],
                    start=(c == 0),
                    stop=(c == 7),
                )
        for t in range(2):
            nc.vector.tensor_copy(qt_sb[t][:, 0:512], qp[t][:])
        kp = [ps.tile([128, 512], f32, tag="a", bufs=2, name=f"kp{t}")
              for t in range(2)]
        for c in range(8):
            for t in range(2):
                nc.tensor.matmul(
                    kp[t][:],
                    wk_sb[:, c * QC + t * 128: c * QC + t * 128 + 128],
                    xT[:, c * SEQ: c * SEQ + 512],
                    start=(c == 0),
                    stop=(c == 7),
                )
        for t in range(2):
            nc.vector.tensor_copy(kt_sb[t][:, 0:512], kp[t][:])

        # ---- main rounds: attention(j=g) woven with proj(g+1)/outproj ----
        # out-projection chunks spread over rounds (ctxt cols of round j feed
        # outproj rc = 4j..4j+3, so round g can emit rc of rounds < g).
        OP_SPREAD = {1: [(rc, n) for rc in (0, 1) for n in range(2)],
                     2: [(rc, n) for rc in (2, 3, 4, 5) for n in range(2)],
                     3: [(rc, n) for rc in (6, 7, 8, 9, 10, 11) for n in range(2)]}
        for g in range(NG):
            j = g
            imax = 4 * j + 3
            npair = (imax + 1) // 2
            filler = []
            if g == 0:
                filler += [(lambda rc=rc: emit_v(0, rc)) for rc in range(4)]
            if g < NG - 1:
                filler += list(proj_chunks(g + 1))
            filler += [
                (lambda rc=rc, n=n: emit_outproj(rc, n))
                for rc, n in OP_SPREAD.get(g, [])
            ]
            steps_total = 2 * npair
            fill_i = 0
            step = 0

            for u in range(2):           # head pair u: heads 2u, 2u+1
                avs = [ps.tile([65, 512], f32, tag="av", bufs=2, name=f"av{p}")
                       for p in range(2)]
                pts = [[], []]           # per parity: list of [128,1024] pair tiles
                for ip in range(npair):
                    i0 = 2 * ip
                    cur = []
                    for p in range(2):
                        sps = ps.tile([128, 1024], f32, tag="a", bufs=2, name="sps")
                        cur.append(sps)
                    for half in range(2):
                        i = i0 + half
                        off = max(0, 128 * i - 512 * j)
                        for p in range(2):
                            o = p * 64
                            nc.tensor.matmul(
                                cur[p][:, half * 512 + off:(half + 1) * 512],
                                kt_sb[u][o:o + 64, i * 128:(i + 1) * 128],
                                qt_sb[u][o:o + 64, j * 512 + off:(j + 1) * 512],
                                start=True,
                                stop=True,
                            )
                    off0 = max(0, 128 * i0 - 512 * j)
                    off1 = max(0, 128 * (i0 + 1) - 512 * j)
                    for p in range(2):
                        pt = ptp.tile([128, 1024], bf16, tag="pt", name="pt")
                        # exp narrowed to the causal-valid region
                        if off1 >= 256:
                            nc.scalar.activation(
                                pt[:, off0:512], cur[p][:, off0:512],
                                EXP, scale=0.125)
                            nc.scalar.activation(
                                pt[:, 512 + off1:1024], cur[p][:, 512 + off1:1024],
                                EXP, scale=0.125)
                        else:
                            nc.scalar.activation(
                                pt[:, off0:1024], cur[p][:, off0:1024],
                                EXP, scale=0.125)
                        for half in range(2):
                            i = i0 + half
                            if i >= 4 * j:
                                off = 128 * i - 512 * j
                                nc.gpsimd.affine_select(
                                    out=pt[:, half * 512 + off:(half + 1) * 512],
                                    in_=pt[:, half * 512 + off:(half + 1) * 512],
                                    compare_op=mybir.AluOpType.is_ge,
                                    fill=0.0,
                                    base=0,
                                    channel_multiplier=-1,
                                    pattern=[[1, 512 - off]],
                                )
                        pts[p].append(pt)
                    if ip >= 1:
                        kp_ = ip - 1
                        for p in range(2):
                            h = 2 * u + p
                            for half in range(2):
                                k = 2 * kp_ + half
                                off = max(0, 128 * k - 512 * j)
                                nc.tensor.matmul(
                                    avs[p][:, off:512],
                                    vones[:, k * VST + h * 65: k * VST + h * 65 + 65],
                                    pts[p][kp_][:, half * 512 + off:(half + 1) * 512],
                                    start=(k == 0),
                                    stop=False,
                                )
                    step += 1
                    want = min(len(filler),
                               (len(filler) * step * 8) // (steps_total * 7))
                    while fill_i < want:
                        filler[fill_i]()
                        fill_i += 1
                # tail AVs for the last pair + immediate per-parity normalize
                kp_ = npair - 1
                for p in range(2):
                    h, o = 2 * u + p, p * 64
                    for half in range(2):
                        k = 2 * kp_ + half
                        off = max(0, 128 * k - 512 * j)
                        nc.tensor.matmul(
                            avs[p][:, off:512],
                            vones[:, k * VST + h * 65: k * VST + h * 65 + 65],
                            pts[p][kp_][:, half * 512 + off:(half + 1) * 512],
                            start=(k == 0),
                            stop=(half == 1),
                        )
                    if g == NG - 1 and u == 1:
                        continue       # chunked normalize below
                    rsrow = nrm.tile([1, 512], f32, tag="rsrow", name="rsrow")
                    nc.vector.tensor_copy(rsrow[:], avs[p][64:65, :])
                    rinv = nrm.tile([1, 512], f32, tag="rinv", name="rinv")
                    nc.vector.reciprocal_approx_fast(rinv[:], rsrow[:])
                    bcast = nrm.tile([64, 512], f32, tag="bcast", name="bcast")
                    nc.gpsimd.partition_broadcast(bcast[:], rinv[:])
                    nc.vector.tensor_mul(
                        ctxt[u][o:o + 64, j * 512:(j + 1) * 512],
                        avs[p][0:64, :],
                        bcast[:],
                    )
                if g == NG - 1 and u == 1:
                    # chunked final normalize: first 128 q-cols (rc12) first so
                    # the first u=1 out-proj matmuls can start ~2us earlier.
                    for lo, hi, tg in ((0, 128, "A"), (128, 512, "B")):
                        w = hi - lo
                        for p in range(2):
                            o = p * 64
                            rs = nrm.tile([1, w], f32, tag=f"rs{tg}", bufs=2,
                                          name=f"rs{tg}")
                            nc.vector.tensor_copy(rs[:], avs[p][64:65, lo:hi])
                            ri = nrm.tile([1, w], f32, tag=f"ri{tg}", bufs=2,
                                          name=f"ri{tg}")
                            nc.vector.reciprocal_approx_fast(ri[:], rs[:])
                            bc = nrm.tile([64, w], f32, tag=f"bc{tg}", bufs=2,
                                          name=f"bc{tg}")
                            nc.gpsimd.partition_broadcast(bc[:], ri[:])
                            nc.vector.tensor_mul(
                                ctxt[1][o:o + 64, 3 * 512 + lo:3 * 512 + hi],
                                avs[p][0:64, lo:hi],
                                bc[:],
                            )
            while fill_i < len(filler):
                filler[fill_i]()
                fill_i += 1

        # ---- final out-projection rc12-15, two-phase: the u=0 halves run on
        # PSUM freed by the last exp ("a") / last woven outproj ("b") — no
        # dependence on the final normalize — keeping the PE busy (and at full
        # p-state) while the chunked normalize completes; u=1 halves interleave
        # as their ctxt[1] cols become ready.
        holdA = {}
        for rc in (12, 13):
            t = ps.tile([128, 1024], f32, tag="a", bufs=2, name=f"hold{rc}")
            for n in range(2):
                nc.tensor.matmul(
                    t[:, n * 512:(n + 1) * 512],
                    ctxt[0][:, rc * 128:(rc + 1) * 128],
                    wo_sb[:, n * 512: n * 512 + 512],
                    start=True,
                    stop=False,
                    skip_group_check=True,
                )
            holdA[rc] = t
        holdB = []
        for n in range(2):
            t = ps.tile([128, 512], f32, tag="b", bufs=2, name=f"hold14_{n}")
            nc.tensor.matmul(
                t[:],
                ctxt[0][:, 14 * 128: 14 * 128 + 128],
                wo_sb[:, n * 512: n * 512 + 512],
                start=True,
                stop=False,
            )
            holdB.append(t)
        # u=1 for rc12 (ctxt[1] cols 1536:1664 ready first)
        for n in range(2):
            nc.tensor.matmul(
                holdA[12][:, n * 512:(n + 1) * 512],
                ctxt[1][:, 12 * 128: 12 * 128 + 128],
                wo_sb[:, DIN + n * 512: DIN + n * 512 + 512],
                start=False,
                stop=True,
                skip_group_check=True,
            )
            drain_out(12, n, holdA[12][:, n * 512:(n + 1) * 512], 1)
        # u=0 for rc15 on the "av" banks (free once the normalize reads done)
        holdV = []
        for n in range(2):
            t = ps.tile([128, 512], f32, tag="av", bufs=2, name=f"hold15_{n}")
            nc.tensor.matmul(
                t[:],
                ctxt[0][:, 15 * 128: 15 * 128 + 128],
                wo_sb[:, n * 512: n * 512 + 512],
                start=True,
                stop=False,
            )
            holdV.append(t)
        for n in range(2):
            nc.tensor.matmul(
                holdA[13][:, n * 512:(n + 1) * 512],
                ctxt[1][:, 13 * 128: 13 * 128 + 128],
                wo_sb[:, DIN + n * 512: DIN + n * 512 + 512],
                start=False,
                stop=True,
                skip_group_check=True,
            )
            drain_out(13, n, holdA[13][:, n * 512:(n + 1) * 512], 0)
        for n in range(2):
            nc.tensor.matmul(
                holdB[n][:],
                ctxt[1][:, 14 * 128: 14 * 128 + 128],
                wo_sb[:, DIN + n * 512: DIN + n * 512 + 512],
                start=False,
                stop=True,
            )
            drain_out(14, n, holdB[n][:], 1)
        for n in range(2):
            nc.tensor.matmul(
                holdV[n][:],
                ctxt[1][:, 15 * 128: 15 * 128 + 128],
                wo_sb[:, DIN + n * 512: DIN + n * 512 + 512],
                start=False,
                stop=True,
            )
            drain_out(15, n, holdV[n][:], n)

    nc.compile()
    return nc


_NC = None


def _get_nc():
    global _NC
    if _NC is None:
        _NC = build_nc()
    return _NC


def kernel(x, W_q, W_k, W_v, W_o, b_o):
    global LAST_RESULTS
    nc = _get_nc()
    bf = ml_dtypes.bfloat16
    x = np.asarray(x, np.float32).astype(bf)
    # pre-transpose per batch (shared by the 4 cores of each batch)
    xT = [np.ascontiguousarray(x[bi].T) for bi in range(2)]
    W_q = np.asarray(W_q, np.float32).astype(bf)
    W_k = np.asarray(W_k, np.float32).astype(bf)
    W_v = np.asarray(W_v, np.float32).astype(bf)
    W_o = np.asarray(W_o, np.float32).astype(bf)
    b_o = np.asarray(b_o, np.float32).reshape(1, DIN)

    def lay_w(w, sl):   # [1024, 256] shard -> [128, 8*256]: t[p, c*256+n] = w[c*128+p, sl][n]
        return np.ascontiguousarray(
            w[:, sl].reshape(8, 128, QC).transpose(1, 0, 2).reshape(128, 8 * QC))

    def lay_wo(w, sl):  # [256, 1024] shard -> [128, 2*1024]
        return np.ascontiguousarray(
            w[sl, :].reshape(2, 128, DIN).transpose(1, 0, 2).reshape(128, 2 * DIN))

    in_maps = []
    for c in range(8):
        bi, g = c // 4, c % 4
        sl = slice(g * QC, (g + 1) * QC)
        in_maps.append({
            "x": xT[bi],
            "wq": lay_w(W_q, sl),
            "wk": lay_w(W_k, sl),
            "wv": lay_w(W_v, sl),
            "wo": lay_wo(W_o, sl),
        })

    res = run_bass_kernel_spmd(nc, in_maps, list(range(8)), trace=TRACE)
    LAST_RESULTS = res
    outs = [np.asarray(r["out"], dtype=np.float32) for r in res.results]
    return np.stack([
        outs[0] + outs[1] + outs[2] + outs[3] + b_o,
        outs[4] + outs[5] + outs[6] + outs[7] + b_o,
    ])


if __name__ == "__main__":
    if "--compile-only" in sys.argv:
        import tempfile
        from concourse.bass_utils import compile_bass_kernel

        nc = build_nc()
        with tempfile.TemporaryDirectory() as td:
            print("walrus compiling...")
            neff = compile_bass_kernel(nc, td)
            print("COMPILE OK", neff)


# revision 6
# speedup vs baseline: 1.0838x; 1.0838x over previous
"""Trainium2 Bass kernel for nn_BaseAttention (causal MHA, b=2, n=2048, d=1024, 16 heads).

Sharding (8 cores): core c handles batch c//4 and heads 4*(c%4)..4*(c%4)+3.
- W_q/W_k/W_v column-sharded (256 cols/core), W_o row-sharded (256 rows/core).
- Each core computes a partial output [2048, 1024] in bf16; host sums the 4
  partials per batch (row-parallel out-projection), adds b_o, stacks batches.

Per-core kernel (bf16 data path, fp32 PSUM accumulation):
  - x is transposed + bf16-cast on the host; weights pre-laid-out on host.
  - startup: c-granular wq/x DMAs + c-outer Q/K round-0 projection so the
    first matmul starts as soon as the first ~200KB lands (cuts DMA head).
  - Q^T/K^T projections emitted transposed; V natural with a ones column per
    head ([V|1] trick: AV yields ctx^T rows 0..63 + softmax row-sum at 64).
  - attention per (head-pair, q-tile j): S^T = K_h @ Q_h^T on PE (even/odd
    heads on disjoint PE row-halves), exp on ACT narrowed to the causal-valid
    region, causal mask via gpsimd affine_select, AV pipelined one i-pair
    behind S, normalization via DVE reciprocal_approx_fast + gpsimd
    partition_broadcast + DVE multiply.
  - projection work of round g+1 and out-projection chunks are woven between
    attention steps (pacing 8/7 so round-boundary qt/kt copies land early);
    out-projection spread over rounds 1-3 to spread PSUM + DMA load.
  - no bias on device: host adds b_o during the partial-sum (free).
  - final out-projection: u=0 halves run on PSUM banks freed by the last exp
    (no wait on the last normalize); the last normalize is chunked so its
    first 128 q-cols are ready ~2us earlier; drains alternate ACT/DVE copies
    (bf16) so the PE never idles and stays at full p-state.
  - all output DMAs on the sync queue (fans out over all 16 DMA engines);
    output in bf16 to halve drain bytes.
"""
import sys, types

sys.path.insert(0, "/opt/trn_rl_repo")


def _install_ntff_shim():
    # antenv.axon_hooks is absent in this image; register the NTFF profile
    # hook via ctypes so run_bass_kernel_spmd(trace=True) works under axon.
    if "antenv.axon_hooks" in sys.modules:
        return
    try:
        sys.path.insert(0, "/root/.axon_site")
        from trn_agent_boot.trn_boot import _ntff_profile_via_ctypes

        hook = _ntff_profile_via_ctypes("/opt/axon/libaxon_pjrt.so")
        mod = types.ModuleType("antenv.axon_hooks")
        mod.get_axon_ntff_profile_hook = lambda: hook
        mod.set_axon_ntff_profile_hook = lambda h: None
        sys.modules["antenv.axon_hooks"] = mod
    except Exception:
        pass


_install_ntff_shim()

import numpy as np
import ml_dtypes
import concourse.bass as bass
import concourse.mybir as mybir
import concourse.tile as tile
from concourse import bacc
from concourse.bass_utils import run_bass_kernel_spmd
from contextlib import ExitStack

f32 = mybir.dt.float32
bf16 = mybir.dt.bfloat16
EXP = mybir.ActivationFunctionType.Exp
COPY = mybir.ActivationFunctionType.Copy

SEQ = 2048          # sequence length
DIN = 1024          # model dim (8 chunks of 128)
QC = 256            # q/k/v cols per core (4 heads x 64)
HD = 64             # head dim
NH = 4              # heads per core
NG = 4              # row groups of 512
VST = NH * 65       # Vones stride per row chunk (4 heads x (64 V + 1 ones))

TRACE = False
LAST_RESULTS = None


def build_nc():
    nc = bacc.Bacc()
    x_d = nc.dram_tensor("x", [DIN, SEQ], bf16, kind="ExternalInput")  # pre-transposed on host
    wq_d = nc.dram_tensor("wq", [128, 8 * QC], bf16, kind="ExternalInput")
    wk_d = nc.dram_tensor("wk", [128, 8 * QC], bf16, kind="ExternalInput")
    wv_d = nc.dram_tensor("wv", [128, 8 * QC], bf16, kind="ExternalInput")
    wo_d = nc.dram_tensor("wo", [128, 2 * DIN], bf16, kind="ExternalInput")
    out_d = nc.dram_tensor("out", [SEQ, DIN], bf16, kind="ExternalOutput")

    with tile.TileContext(nc, pool_alloc_mode="queue") as tc, ExitStack() as ctx:
        wr = ctx.enter_context(tc.tile_pool(name="wr", bufs=1))
        cst = ctx.enter_context(tc.tile_pool(name="cst", bufs=1))
        big = ctx.enter_context(tc.tile_pool(name="big", bufs=1))
        ptp = ctx.enter_context(tc.tile_pool(name="ptp", bufs=8))
        nrm = ctx.enter_context(tc.tile_pool(name="nrm", bufs=3))
        ob = ctx.enter_context(tc.tile_pool(name="ob", bufs=6))
        ps = ctx.enter_context(tc.tile_pool(name="ps", bufs=1, space="PSUM"))

        # ---- DMAs, ordered by first use (weights pre-laid-out on host).
        # weights on the scalar HWDGE queue, x on sync: parallel issue.
        # First pieces tiny so the first Q-proj matmul can start ~9.5us in.
        wq_sb = wr.tile([128, 8 * QC], bf16, name="wq_sb")
        nc.scalar.dma_start(wq_sb[:, 0:QC], wq_d[:, 0:QC])          # c0
        nc.scalar.dma_start(wq_sb[:, QC:4 * QC], wq_d[:, QC:4 * QC])
        nc.scalar.dma_start(wq_sb[:, 4 * QC:], wq_d[:, 4 * QC:])

        # x arrives pre-transposed: straight contiguous copy DMAs at full BW.
        # xT[p, c*2048+r] = x[r, c*128+p] = x_d[c*128+p, r]
        xT = big.tile([128, 8 * SEQ], bf16)
        xview = xT[:].rearrange("p (c r) -> p c r", r=SEQ)
        dview = x_d.rearrange("(c p) r -> p c r", p=128)
        nc.sync.dma_start(xview[:, 0:1, 0:512], dview[:, 0:1, 0:512])
        nc.sync.dma_start(xview[:, 1:2, 0:512], dview[:, 1:2, 0:512])
        nc.sync.dma_start(xview[:, 2:4, 0:512], dview[:, 2:4, 0:512])
        nc.sync.dma_start(xview[:, 4:8, 0:512], dview[:, 4:8, 0:512])

        wk_sb = wr.tile([128, 8 * QC], bf16, name="wk_sb")
        nc.scalar.dma_start(wk_sb[:, 0:4 * QC], wk_d[:, 0:4 * QC])
        nc.scalar.dma_start(wk_sb[:, 4 * QC:], wk_d[:, 4 * QC:])
        wv_sb = wr.tile([128, 8 * QC], bf16, name="wv_sb")
        nc.scalar.dma_start(wv_sb[:], wv_d[:])

        for g in range(1, NG):
            nc.sync.dma_start(
                xview[:, :, g * 512:(g + 1) * 512],
                dview[:, :, g * 512:(g + 1) * 512],
            )
        wo_sb = cst.tile([128, 2 * DIN], bf16)
        nc.scalar.dma_start(wo_sb[:], wo_d[:])

        # dummy partition_broadcast: triggers the one-time gpsimd library
        # load (~7us) during the DMA head instead of mid-kernel.
        dummy_src = cst.tile([1, 8], f32)
        nc.vector.memset(dummy_src[:], 0.0)
        dummy_dst = cst.tile([64, 8], f32)
        nc.gpsimd.partition_broadcast(dummy_dst[:], dummy_src[:])

        # ---- persistent activations ----
        qt_sb = [big.tile([128, SEQ], bf16, name=f"qt{t}") for t in range(2)]
        kt_sb = [big.tile([128, SEQ], bf16, name=f"kt{t}") for t in range(2)]
        vones = big.tile([128, 16 * VST], bf16)
        ctxt = [big.tile([128, SEQ], bf16, name=f"ctxt{t}") for t in range(2)]

        vview = vones.rearrange("p (r h e) -> p r h e", h=NH, e=65)
        nc.vector.memset(vview[:, :, :, 64], 1.0)

        # ---- emission helpers ----
        def emit_qk(g, t, wt, dst):
            prj = ps.tile([128, 512], f32, tag="b", bufs=2, name="prj")
            for c in range(8):
                nc.tensor.matmul(
                    prj[:],
                    wt[:, c * QC + t * 128: c * QC + t * 128 + 128],
                    xT[:, c * SEQ + g * 512: c * SEQ + g * 512 + 512],
                    start=(c == 0),
                    stop=(c == 7),
                )
            nc.vector.tensor_copy(dst[t][:, g * 512:(g + 1) * 512], prj[:])

        def emit_v(g, rc):
            rcg = 4 * g + rc
            vps = ps.tile([128, 256], f32, tag="b", bufs=2, name="vps")
            for c in range(8):
                nc.tensor.matmul(
                    vps[:],
                    xT[:, c * SEQ + rcg * 128: c * SEQ + rcg * 128 + 128],
                    wv_sb[:, c * QC:(c + 1) * QC],
                    start=(c == 0),
                    stop=(c == 7),
                )
            nc.vector.tensor_copy(
                vview[:, rcg, :, 0:64],
                vps[:].rearrange("p (h e) -> p h e", e=HD),
            )

        def drain_out(rc, n, src, eng):
            osb = ob.tile([128, 512], bf16, tag="o", name="osb")
            if eng == 0:
                nc.vector.tensor_copy(osb[:], src)
            else:
                nc.scalar.activation(osb[:], src, COPY)
            nc.sync.dma_start(
                out_d[rc * 128:(rc + 1) * 128, n * 512:(n + 1) * 512], osb[:]
            )

        def emit_outproj(rc, n, tag="b"):
            ops = ps.tile([128, 512], f32, tag=tag, bufs=2, name="ops")
            for u in range(2):
                nc.tensor.matmul(
                    ops[:],
                    ctxt[u][:, rc * 128:(rc + 1) * 128],
                    wo_sb[:, u * DIN + n * 512: u * DIN + n * 512 + 512],
                    start=(u == 0),
                    stop=(u == 1),
                )
            drain_out(rc, n, ops[:], 0)

        def proj_chunks(g):
            for t in range(2):
                yield lambda t=t: emit_qk(g, t, wq_sb, qt_sb)
                yield lambda t=t: emit_qk(g, t, wk_sb, kt_sb)
            for rc in range(4):
                yield lambda rc=rc: emit_v(g, rc)

        # ---- round 0 Q/K projection, c-outer so the first matmul needs only
        # the first wq/x chunks; V(g0) is woven into j=0 as filler.
        qp = [ps.tile([128, 512], f32, tag="b", bufs=2, name=f"qp{t}")
              for t in range(2)]
        for c in range(8):
            for t in range(2):
                nc.tensor.matmul(
                    qp[t][:],
                    wq_sb[:, c * QC + t * 128: c * QC + t * 128 + 128],
                    xT[:, c * SEQ: c * SEQ + 512],
                    start=(c == 0),
                    stop=(c == 7),
                )
        for t in range(2):
            nc.vector.tensor_copy(qt_sb[t][:, 0:512], qp[t][:])
        kp = [ps.tile([128, 512], f32, tag="a", bufs=2, name=f"kp{t}")
              for t in range(2)]
        for c in range(8):
            for t in range(2):
                nc.tensor.matmul(
                    kp[t][:],
                    wk_sb[:, c * QC + t * 128: c * QC + t * 128 + 128],
                    xT[:, c * SEQ: c * SEQ + 512],
                    start=(c == 0),
                    stop=(c == 7),
                )
        for t in range(2):
            nc.vector.tensor_copy(kt_sb[t][:, 0:512], kp[t][:])

        # ---- main rounds: attention(j=g) woven with proj(g+1)/outproj ----
        # out-projection chunks spread over rounds (ctxt cols of round j feed
        # outproj rc = 4j..4j+3, so round g can emit rc of rounds < g).
        OP_SPREAD = {1: [(rc, n) for rc in (0, 1) for n in range(2)],
                     2: [(rc, n) for rc in (2, 3, 4, 5) for n in range(2)],
                     3: [(rc, n) for rc in (6, 7, 8, 9, 10, 11) for n in range(2)]}
        for g in range(NG):
            j = g
            imax = 4 * j + 3
            npair = (imax + 1) // 2
            filler = []
            if g == 0:
                filler += [(lambda rc=rc: emit_v(0, rc)) for rc in range(4)]
            if g < NG - 1:
                filler += list(proj_chunks(g + 1))
            filler += [
                (lambda rc=rc, n=n: emit_outproj(rc, n))
                for rc, n in OP_SPREAD.get(g, [])
            ]
            steps_total = 2 * npair
            fill_i = 0
            step = 0

            for u in range(2):           # head pair u: heads 2u, 2u+1
                avs = [ps.tile([65, 512], f32, tag="av", bufs=2, name=f"av{p}")
                       for p in range(2)]
                pts = [[], []]           # per parity: list of [128,1024] pair tiles
                for ip in range(npair):
                    i0 = 2 * ip
                    cur = []
                    for p in range(2):
                        sps = ps.tile([128, 1024], f32, tag="a", bufs=2, name="sps")
                        cur.append(sps)
                    for half in range(2):
                        i = i0 + half
                        off = max(0, 128 * i - 512 * j)
                        for p in range(2):
                            o = p * 64
                            nc.tensor.matmul(
                                cur[p][:, half * 512 + off:(half + 1) * 512],
                                kt_sb[u][o:o + 64, i * 128:(i + 1) * 128],
                                qt_sb[u][o:o + 64, j * 512 + off:(j + 1) * 512],
                                start=True,
                                stop=True,
                            )
                    off0 = max(0, 128 * i0 - 512 * j)
                    off1 = max(0, 128 * (i0 + 1) - 512 * j)
                    for p in range(2):
                        pt = ptp.tile([128, 1024], bf16, tag="pt", name="pt")
                        # exp narrowed to the causal-valid region
                        if off1 >= 256:
                            nc.scalar.activation(
                                pt[:, off0:512], cur[p][:, off0:512],
                                EXP, scale=0.125)
                            nc.scalar.activation(
                                pt[:, 512 + off1:1024], cur[p][:, 512 + off1:1024],
                                EXP, scale=0.125)
                        else:
                            nc.scalar.activation(
                                pt[:, off0:1024], cur[p][:, off0:1024],
                                EXP, scale=0.125)
                        for half in range(2):
                            i = i0 + half
                            if i >= 4 * j:
                                off = 128 * i - 512 * j
                                nc.gpsimd.affine_select(
                                    out=pt[:, half * 512 + off:(half + 1) * 512],
                                    in_=pt[:, half * 512 + off:(half + 1) * 512],
                                    compare_op=mybir.AluOpType.is_ge,
                                    fill=0.0,
                                    base=0,
                                    channel_multiplier=-1,
                                    pattern=[[1, 512 - off]],
                                )
                        pts[p].append(pt)
                    if ip >= 1:
                        kp_ = ip - 1
                        for p in range(2):
                            h = 2 * u + p
                            for half in range(2):
                                k = 2 * kp_ + half
                                off = max(0, 128 * k - 512 * j)
                                nc.tensor.matmul(
                                    avs[p][:, off:512],
                                    vones[:, k * VST + h * 65: k * VST + h * 65 + 65],
                                    pts[p][kp_][:, half * 512 + off:(half + 1) * 512],
                                    start=(k == 0),
                                    stop=False,
                                )
                    step += 1
                    want = min(len(filler),
                               (len(filler) * step * 8) // (steps_total * 7))
                    while fill_i < want:
                        filler[fill_i]()
                        fill_i += 1
                # tail AVs for the last pair + immediate per-parity normalize
                kp_ = npair - 1
                for p in range(2):
                    h, o = 2 * u + p, p * 64
                    for half in range(2):
                        k = 2 * kp_ + half
                        off = max(0, 128 * k - 512 * j)
                        nc.tensor.matmul(
                            avs[p][:, off:512],
                            vones[:, k * VST + h * 65: k * VST + h * 65 + 65],
                            pts[p][kp_][:, half * 512 + off:(half + 1) * 512],
                            start=(k == 0),
                            stop=(half == 1),
                        )
                    if g == NG - 1 and u == 1:
                        continue       # chunked normalize below
                    rsrow = nrm.tile([1, 512], f32, tag="rsrow", name="rsrow")
                    nc.vector.tensor_copy(rsrow[:], avs[p][64:65, :])
                    rinv = nrm.tile([1, 512], f32, tag="rinv", name="rinv")
                    nc.vector.reciprocal_approx_fast(rinv[:], rsrow[:])
                    bcast = nrm.tile([64, 512], f32, tag="bcast", name="bcast")
                    nc.gpsimd.partition_broadcast(bcast[:], rinv[:])
                    nc.vector.tensor_mul(
                        ctxt[u][o:o + 64, j * 512:(j + 1) * 512],
                        avs[p][0:64, :],
                        bcast[:],
                    )
                if g == NG - 1 and u == 1:
                    # chunked final normalize: first 128 q-cols (rc12) first so
                    # the first u=1 out-proj matmuls can start ~2us earlier.
                    for lo, hi, tg in ((0, 128, "A"), (128, 512, "B")):
                        w = hi - lo
                        for p in range(2):
                            o = p * 64
                            rs = nrm.tile([1, w], f32, tag=f"rs{tg}", bufs=2,
                                          name=f"rs{tg}")
                            nc.vector.tensor_copy(rs[:], avs[p][64:65, lo:hi])
                            ri = nrm.tile([1, w], f32, tag=f"ri{tg}", bufs=2,
                                          name=f"ri{tg}")
                            nc.vector.reciprocal_approx_fast(ri[:], rs[:])
                            bc = nrm.tile([64, w], f32, tag=f"bc{tg}", bufs=2,
                                          name=f"bc{tg}")
                            nc.gpsimd.partition_broadcast(bc[:], ri[:])
                            nc.vector.tensor_mul(
                                ctxt[1][o:o + 64, 3 * 512 + lo:3 * 512 + hi],
                                avs[p][0:64, lo:hi],
                                bc[:],
                            )
            while fill_i < len(filler):
                filler[fill_i]()
                fill_i += 1

        # ---- final out-projection rc12-15, two-phase: the u=0 halves run on
        # PSUM freed by the last exp ("a") / last woven outproj ("b") — no
        # dependence on the final normalize — keeping the PE busy (and at full
        # p-state) while the chunked normalize completes; u=1 halves interleave
        # as their ctxt[1] cols become ready.
        holdA = {}
        for rc in (12, 13):
            t = ps.tile([128, 1024], f32, tag="a", bufs=2, name=f"hold{rc}")
            for n in range(2):
                nc.tensor.matmul(
                    t[:, n * 512:(n + 1) * 512],
                    ctxt[0][:, rc * 128:(rc + 1) * 128],
                    wo_sb[:, n * 512: n * 512 + 512],
                    start=True,
                    stop=False,
                    skip_group_check=True,
                )
            holdA[rc] = t
        holdB = []
        for n in range(2):
            t = ps.tile([128, 512], f32, tag="b", bufs=2, name=f"hold14_{n}")
            nc.tensor.matmul(
                t[:],
                ctxt[0][:, 14 * 128: 14 * 128 + 128],
                wo_sb[:, n * 512: n * 512 + 512],
                start=True,
                stop=False,
            )
            holdB.append(t)
        # u=1 for rc12 (ctxt[1] cols 1536:1664 ready first)
        for n in range(2):
            nc.tensor.matmul(
                holdA[12][:, n * 512:(n + 1) * 512],
                ctxt[1][:, 12 * 128: 12 * 128 + 128],
                wo_sb[:, DIN + n * 512: DIN + n * 512 + 512],
                start=False,
                stop=True,
                skip_group_check=True,
            )
            drain_out(12, n, holdA[12][:, n * 512:(n + 1) * 512], 1)
        # u=0 for rc15 on the "av" banks (free once the normalize reads done)
        holdV = []
        for n in range(2):
            t = ps.tile([128, 512], f32, tag="av", bufs=2, name=f"hold15_{n}")
            nc.tensor.matmul(
                t[:],
                ctxt[0][:, 15 * 128: 15 * 128 + 128],
                wo_sb[:, n * 512: n * 512 + 512],
                start=True,
                stop=False,
            )
            holdV.append(t)
        for n in range(2):
            nc.tensor.matmul(
                holdA[13][:, n * 512:(n + 1) * 512],
                ctxt[1][:, 13 * 128: 13 * 128 + 128],
                wo_sb[:, DIN + n * 512: DIN + n * 512 + 512],
                start=False,
                stop=True,
                skip_group_check=True,
            )
            drain_out(13, n, holdA[13][:, n * 512:(n + 1) * 512], 1)
        for n in range(2):
            nc.tensor.matmul(
                holdB[n][:],
                ctxt[1][:, 14 * 128: 14 * 128 + 128],
                wo_sb[:, DIN + n * 512: DIN + n * 512 + 512],
                start=False,
                stop=True,
            )
            drain_out(14, n, holdB[n][:], 0)
        for n in range(2):
            nc.tensor.matmul(
                holdV[n][:],
                ctxt[1][:, 15 * 128: 15 * 128 + 128],
                wo_sb[:, DIN + n * 512: DIN + n * 512 + 512],
                start=False,
                stop=True,
            )
            drain_out(15, n, holdV[n][:], n)

    nc.compile()
    return nc


_NC = None


def _get_nc():
    global _NC
    if _NC is None:
        _NC = build_nc()
    return _NC


def kernel(x, W_q, W_k, W_v, W_o, b_o):
    global LAST_RESULTS
    nc = _get_nc()
    bf = ml_dtypes.bfloat16
    x = np.asarray(x, np.float32).astype(bf)
    # pre-transpose per batch (shared by the 4 cores of each batch)
    xT = [np.ascontiguousarray(x[bi].T) for bi in range(2)]
    W_q = np.asarray(W_q, np.float32).astype(bf)
    W_k = np.asarray(W_k, np.float32).astype(bf)
    W_v = np.asarray(W_v, np.float32).astype(bf)
    W_o = np.asarray(W_o, np.float32).astype(bf)
    b_o = np.asarray(b_o, np.float32).reshape(1, DIN)

    def lay_w(w, sl):   # [1024, 256] shard -> [128, 8*256]: t[p, c*256+n] = w[c*128+p, sl][n]
        return np.ascontiguousarray(
            w[:, sl].reshape(8, 128, QC).transpose(1, 0, 2).reshape(128, 8 * QC))

    def lay_wo(w, sl):  # [256, 1024] shard -> [128, 2*1024]
        return np.ascontiguousarray(
            w[sl, :].reshape(2, 128, DIN).transpose(1, 0, 2).reshape(128, 2 * DIN))

    in_maps = []
    for c in range(8):
        bi, g = c // 4, c % 4
        sl = slice(g * QC, (g + 1) * QC)
        in_maps.append({
            "x": xT[bi],
            "wq": lay_w(W_q, sl),
            "wk": lay_w(W_k, sl),
            "wv": lay_w(W_v, sl),
            "wo": lay_wo(W_o, sl),
        })

    res = run_bass_kernel_spmd(nc, in_maps, list(range(8)), trace=TRACE)
    LAST_RESULTS = res
    outs = [np.asarray(r["out"], dtype=np.float32) for r in res.results]
    return np.stack([
        outs[0] + outs[1] + outs[2] + outs[3] + b_o,
        outs[4] + outs[5] + outs[6] + outs[7] + b_o,
    ])


if __name__ == "__main__":
    if "--compile-only" in sys.argv:
        import tempfile
        from concourse.bass_utils import compile_bass_kernel

        nc = build_nc()
        with tempfile.TemporaryDirectory() as td:
            print("walrus compiling...")
            neff = compile_bass_kernel(nc, td)
            print("COMPILE OK", neff)


# revision 8
# speedup vs baseline: 1.1001x; 1.0150x over previous
"""Trainium2 Bass kernel for nn_BaseAttention (causal MHA, b=2, n=2048, d=1024, 16 heads).

Sharding (8 cores): core c handles batch c//4 and heads 4*(c%4)..4*(c%4)+3.
- W_q/W_k/W_v column-sharded (256 cols/core), W_o row-sharded (256 rows/core).
- Each core computes a partial output [2048, 1024] in bf16; host sums the 4
  partials per batch (row-parallel out-projection), adds b_o, stacks batches.

Per-core kernel (bf16 data path, fp32 PSUM accumulation):
  - x is transposed + bf16-cast on the host; weights pre-laid-out on host.
  - startup: c-granular wq/x DMAs + c-outer Q/K round-0 projection so the
    first matmul starts as soon as the first ~200KB lands (cuts DMA head).
  - Q^T/K^T projections emitted transposed; V natural with a ones column per
    head ([V|1] trick: AV yields ctx^T rows 0..63 + softmax row-sum at 64).
  - attention per (head-pair, q-tile j): S^T = K_h @ Q_h^T on PE (even/odd
    heads on disjoint PE row-halves), exp on ACT narrowed to the causal-valid
    region, causal mask via gpsimd affine_select, AV pipelined one i-pair
    behind S, normalization via DVE reciprocal_approx_fast + gpsimd
    partition_broadcast + DVE multiply.
  - projection work of round g+1 and out-projection chunks are woven between
    attention steps (pacing 8/7 so round-boundary qt/kt copies land early);
    out-projection spread over rounds 1-3 to spread PSUM + DMA load.
  - no bias on device: host adds b_o during the partial-sum (free).
  - final out-projection: u=0 halves run on PSUM banks freed by the last exp
    (no wait on the last normalize); the last normalize is chunked so its
    first 128 q-cols are ready ~2us earlier; drains alternate ACT/DVE copies
    (bf16) so the PE never idles and stays at full p-state.
  - all output DMAs on the sync queue (fans out over all 16 DMA engines);
    output in bf16 to halve drain bytes.
"""
import sys, types

sys.path.insert(0, "/opt/trn_rl_repo")


def _install_ntff_shim():
    # antenv.axon_hooks is absent in this image; register the NTFF profile
    # hook via ctypes so run_bass_kernel_spmd(trace=True) works under axon.
    if "antenv.axon_hooks" in sys.modules:
        return
    try:
        sys.path.insert(0, "/root/.axon_site")
        from trn_agent_boot.trn_boot import _ntff_profile_via_ctypes

        hook = _ntff_profile_via_ctypes("/opt/axon/libaxon_pjrt.so")
        mod = types.ModuleType("antenv.axon_hooks")
        mod.get_axon_ntff_profile_hook = lambda: hook
        mod.set_axon_ntff_profile_hook = lambda h: None
        sys.modules["antenv.axon_hooks"] = mod
    except Exception:
        pass


_install_ntff_shim()

import numpy as np
import ml_dtypes
import concourse.bass as bass
import concourse.mybir as mybir
import concourse.tile as tile
from concourse import bacc
from concourse.bass_utils import run_bass_kernel_spmd
from contextlib import ExitStack

f32 = mybir.dt.float32
bf16 = mybir.dt.bfloat16
EXP = mybir.ActivationFunctionType.Exp
COPY = mybir.ActivationFunctionType.Copy

SEQ = 2048          # sequence length
DIN = 1024          # model dim (8 chunks of 128)
QC = 256            # q/k/v cols per core (4 heads x 64)
HD = 64             # head dim
NH = 4              # heads per core
NG = 4              # row groups of 512
VST = NH * 65       # Vones stride per row chunk (4 heads x (64 V + 1 ones))

TRACE = False
LAST_RESULTS = None


def build_nc():
    nc = bacc.Bacc()
    x_d = nc.dram_tensor("x", [DIN, SEQ], bf16, kind="ExternalInput")  # pre-transposed on host
    wq_d = nc.dram_tensor("wq", [128, 8 * QC], bf16, kind="ExternalInput")
    wk_d = nc.dram_tensor("wk", [128, 8 * QC], bf16, kind="ExternalInput")
    wv_d = nc.dram_tensor("wv", [128, 8 * QC], bf16, kind="ExternalInput")
    wo_d = nc.dram_tensor("wo", [128, 2 * DIN], bf16, kind="ExternalInput")
    out_d = nc.dram_tensor("out", [SEQ, DIN], bf16, kind="ExternalOutput")

    with tile.TileContext(nc, pool_alloc_mode="queue") as tc, ExitStack() as ctx:
        wr = ctx.enter_context(tc.tile_pool(name="wr", bufs=1))
        cst = ctx.enter_context(tc.tile_pool(name="cst", bufs=1))
        big = ctx.enter_context(tc.tile_pool(name="big", bufs=1))
        ptp = ctx.enter_context(tc.tile_pool(name="ptp", bufs=8))
        nrm = ctx.enter_context(tc.tile_pool(name="nrm", bufs=3))
        ob = ctx.enter_context(tc.tile_pool(name="ob", bufs=6))
        ps = ctx.enter_context(tc.tile_pool(name="ps", bufs=1, space="PSUM"))

        # ---- DMAs, ordered by first use (weights pre-laid-out on host).
        # weights on the scalar HWDGE queue, x on sync: parallel issue.
        # First pieces tiny so the first Q-proj matmul can start ~9.5us in.
        wq_sb = wr.tile([128, 8 * QC], bf16, name="wq_sb")
        nc.scalar.dma_start(wq_sb[:, 0:QC], wq_d[:, 0:QC])          # c0
        nc.scalar.dma_start(wq_sb[:, QC:4 * QC], wq_d[:, QC:4 * QC])
        nc.scalar.dma_start(wq_sb[:, 4 * QC:], wq_d[:, 4 * QC:])

        # x arrives pre-transposed: straight contiguous copy DMAs at full BW.
        # xT[p, c*2048+r] = x[r, c*128+p] = x_d[c*128+p, r]
        xT = big.tile([128, 8 * SEQ], bf16)
        xview = xT[:].rearrange("p (c r) -> p c r", r=SEQ)
        dview = x_d.rearrange("(c p) r -> p c r", p=128)
        nc.sync.dma_start(xview[:, 0:1, 0:512], dview[:, 0:1, 0:512])
        nc.sync.dma_start(xview[:, 1:2, 0:512], dview[:, 1:2, 0:512])
        nc.sync.dma_start(xview[:, 2:4, 0:512], dview[:, 2:4, 0:512])
        nc.sync.dma_start(xview[:, 4:8, 0:512], dview[:, 4:8, 0:512])

        wk_sb = wr.tile([128, 8 * QC], bf16, name="wk_sb")
        nc.scalar.dma_start(wk_sb[:, 0:4 * QC], wk_d[:, 0:4 * QC])
        nc.scalar.dma_start(wk_sb[:, 4 * QC:], wk_d[:, 4 * QC:])
        wv_sb = wr.tile([128, 8 * QC], bf16, name="wv_sb")
        nc.scalar.dma_start(wv_sb[:], wv_d[:])

        for g in range(1, NG):
            nc.sync.dma_start(
                xview[:, :, g * 512:(g + 1) * 512],
                dview[:, :, g * 512:(g + 1) * 512],
            )
        wo_sb = cst.tile([128, 2 * DIN], bf16)
        nc.scalar.dma_start(wo_sb[:], wo_d[:])

        # dummy partition_broadcast: triggers the one-time gpsimd library
        # load (~7us) during the DMA head instead of mid-kernel.
        dummy_src = cst.tile([1, 8], f32)
        nc.vector.memset(dummy_src[:], 0.0)
        dummy_dst = cst.tile([64, 8], f32)
        nc.gpsimd.partition_broadcast(dummy_dst[:], dummy_src[:])

        # ---- persistent activations ----
        qt_sb = [big.tile([128, SEQ], bf16, name=f"qt{t}") for t in range(2)]
        kt_sb = [big.tile([128, SEQ], bf16, name=f"kt{t}") for t in range(2)]
        vones = big.tile([128, 16 * VST], bf16)
        ctxt = [big.tile([128, SEQ], bf16, name=f"ctxt{t}") for t in range(2)]

        vview = vones.rearrange("p (r h e) -> p r h e", h=NH, e=65)
        nc.vector.memset(vview[:, :, :, 64], 1.0)

        # ---- emission helpers ----
        def emit_qk(g, t, wt, dst):
            prj = ps.tile([128, 512], f32, tag="b", bufs=2, name="prj")
            for c in range(8):
                nc.tensor.matmul(
                    prj[:],
                    wt[:, c * QC + t * 128: c * QC + t * 128 + 128],
                    xT[:, c * SEQ + g * 512: c * SEQ + g * 512 + 512],
                    start=(c == 0),
                    stop=(c == 7),
                )
            nc.vector.tensor_copy(dst[t][:, g * 512:(g + 1) * 512], prj[:])

        def emit_v(g, rc):
            rcg = 4 * g + rc
            vps = ps.tile([128, 256], f32, tag="b", bufs=2, name="vps")
            for c in range(8):
                nc.tensor.matmul(
                    vps[:],
                    xT[:, c * SEQ + rcg * 128: c * SEQ + rcg * 128 + 128],
                    wv_sb[:, c * QC:(c + 1) * QC],
                    start=(c == 0),
                    stop=(c == 7),
                )
            nc.vector.tensor_copy(
                vview[:, rcg, :, 0:64],
                vps[:].rearrange("p (h e) -> p h e", e=HD),
            )

        def drain_out(rc, n, src, eng):
            osb = ob.tile([128, 512], bf16, tag="o", name="osb")
            if eng == 0:
                nc.vector.tensor_copy(osb[:], src)
            else:
                nc.scalar.activation(osb[:], src, COPY)
            nc.sync.dma_start(
                out_d[rc * 128:(rc + 1) * 128, n * 512:(n + 1) * 512], osb[:]
            )

        def emit_outproj(rc, n, tag="b"):
            ops = ps.tile([128, 512], f32, tag=tag, bufs=2, name="ops")
            for u in range(2):
                nc.tensor.matmul(
                    ops[:],
                    ctxt[u][:, rc * 128:(rc + 1) * 128],
                    wo_sb[:, u * DIN + n * 512: u * DIN + n * 512 + 512],
                    start=(u == 0),
                    stop=(u == 1),
                )
            drain_out(rc, n, ops[:], 0)

        def proj_chunks(g):
            for t in range(2):
                yield lambda t=t: emit_qk(g, t, wq_sb, qt_sb)
                yield lambda t=t: emit_qk(g, t, wk_sb, kt_sb)
            for rc in range(4):
                yield lambda rc=rc: emit_v(g, rc)

        # ---- round 0 Q/K projection, c-outer so the first matmul needs only
        # the first wq/x chunks; V(g0) is woven into j=0 as filler.
        qp = [ps.tile([128, 512], f32, tag="b", bufs=2, name=f"qp{t}")
              for t in range(2)]
        for c in range(8):
            for t in range(2):
                nc.tensor.matmul(
                    qp[t][:],
                    wq_sb[:, c * QC + t * 128: c * QC + t * 128 + 128],
                    xT[:, c * SEQ: c * SEQ + 512],
                    start=(c == 0),
                    stop=(c == 7),
                )
        for t in range(2):
            nc.vector.tensor_copy(qt_sb[t][:, 0:512], qp[t][:])
        kp = [ps.tile([128, 512], f32, tag="a", bufs=2, name=f"kp{t}")
              for t in range(2)]
        for c in range(8):
            for t in range(2):
                nc.tensor.matmul(
                    kp[t][:],
                    wk_sb[:, c * QC + t * 128: c * QC + t * 128 + 128],
                    xT[:, c * SEQ: c * SEQ + 512],
                    start=(c == 0),
                    stop=(c == 7),
                )
        for t in range(2):
            nc.vector.tensor_copy(kt_sb[t][:, 0:512], kp[t][:])

        # ---- main rounds: attention(j=g) woven with proj(g+1)/outproj ----
        # out-projection chunks spread over rounds (ctxt cols of round j feed
        # outproj rc = 4j..4j+3, so round g can emit rc of rounds < g).
        OP_SPREAD = {1: [(rc, n) for rc in (0, 1) for n in range(2)],
                     2: [(rc, n) for rc in (2, 3, 4, 5) for n in range(2)],
                     3: [(rc, n) for rc in (6, 7, 8, 9, 10, 11) for n in range(2)]}
        for g in range(NG):
            j = g
            imax = 4 * j + 3
            npair = (imax + 1) // 2
            filler = []
            if g == 0:
                filler += [(lambda rc=rc: emit_v(0, rc)) for rc in range(4)]
            if g < NG - 1:
                filler += list(proj_chunks(g + 1))
            filler += [
                (lambda rc=rc, n=n: emit_outproj(rc, n))
                for rc, n in OP_SPREAD.get(g, [])
            ]
            steps_total = 2 * npair
            fill_i = 0
            step = 0

            for u in range(2):           # head pair u: heads 2u, 2u+1
                avs = [ps.tile([65, 512], f32, tag="av", bufs=2, name=f"av{p}")
                       for p in range(2)]
                pts = [[], []]           # per parity: list of [128,1024] pair tiles
                for ip in range(npair):
                    i0 = 2 * ip
                    cur = []
                    for p in range(2):
                        sps = ps.tile([128, 1024], f32, tag="a", bufs=2, name="sps")
                        cur.append(sps)
                    for half in range(2):
                        i = i0 + half
                        off = max(0, 128 * i - 512 * j)
                        for p in range(2):
                            o = p * 64
                            nc.tensor.matmul(
                                cur[p][:, half * 512 + off:(half + 1) * 512],
                                kt_sb[u][o:o + 64, i * 128:(i + 1) * 128],
                                qt_sb[u][o:o + 64, j * 512 + off:(j + 1) * 512],
                                start=True,
                                stop=True,
                            )
                    off0 = max(0, 128 * i0 - 512 * j)
                    off1 = max(0, 128 * (i0 + 1) - 512 * j)
                    for p in range(2):
                        pt = ptp.tile([128, 1024], bf16, tag="pt", name="pt")
                        # exp narrowed to the causal-valid region
                        if off1 >= 256:
                            nc.scalar.activation(
                                pt[:, off0:512], cur[p][:, off0:512],
                                EXP, scale=0.125)
                            nc.scalar.activation(
                                pt[:, 512 + off1:1024], cur[p][:, 512 + off1:1024],
                                EXP, scale=0.125)
                        else:
                            nc.scalar.activation(
                                pt[:, off0:1024], cur[p][:, off0:1024],
                                EXP, scale=0.125)
                        for half in range(2):
                            i = i0 + half
                            if i >= 4 * j:
                                off = 128 * i - 512 * j
                                nc.gpsimd.affine_select(
                                    out=pt[:, half * 512 + off:(half + 1) * 512],
                                    in_=pt[:, half * 512 + off:(half + 1) * 512],
                                    compare_op=mybir.AluOpType.is_ge,
                                    fill=0.0,
                                    base=0,
                                    channel_multiplier=-1,
                                    pattern=[[1, 512 - off]],
                                )
                        pts[p].append(pt)
                    if ip >= 1:
                        kp_ = ip - 1
                        for p in range(2):
                            h = 2 * u + p
                            for half in range(2):
                                k = 2 * kp_ + half
                                off = max(0, 128 * k - 512 * j)
                                nc.tensor.matmul(
                                    avs[p][:, off:512],
                                    vones[:, k * VST + h * 65: k * VST + h * 65 + 65],
                                    pts[p][kp_][:, half * 512 + off:(half + 1) * 512],
                                    start=(k == 0),
                                    stop=False,
                                )
                    step += 1
                    want = min(len(filler),
                               (len(filler) * step * 8) // (steps_total * 7))
                    while fill_i < want:
                        filler[fill_i]()
                        fill_i += 1
                # tail AVs for the last pair + immediate per-parity normalize
                kp_ = npair - 1
                for p in range(2):
                    h, o = 2 * u + p, p * 64
                    for half in range(2):
                        k = 2 * kp_ + half
                        off = max(0, 128 * k - 512 * j)
                        nc.tensor.matmul(
                            avs[p][:, off:512],
                            vones[:, k * VST + h * 65: k * VST + h * 65 + 65],
                            pts[p][kp_][:, half * 512 + off:(half + 1) * 512],
                            start=(k == 0),
                            stop=(half == 1),
                        )
                    if g == NG - 1 and u == 1:
                        continue       # chunked normalize below
                    rsrow = nrm.tile([1, 512], f32, tag="rsrow", name="rsrow")
                    nc.vector.tensor_copy(rsrow[:], avs[p][64:65, :])
                    rinv = nrm.tile([1, 512], f32, tag="rinv", name="rinv")
                    nc.vector.reciprocal_approx_fast(rinv[:], rsrow[:])
                    bcast = nrm.tile([64, 512], f32, tag="bcast", name="bcast")
                    nc.gpsimd.partition_broadcast(bcast[:], rinv[:])
                    nc.vector.tensor_mul(
                        ctxt[u][o:o + 64, j * 512:(j + 1) * 512],
                        avs[p][0:64, :],
                        bcast[:],
                    )
                if g == NG - 1 and u == 1:
                    # chunked final normalize: halves [0:256] (rc12/13) and
                    # [256:512] (rc14/15) so the first u=1 out-proj matmuls
                    # can start ~2us earlier.
                    for lo, hi, tg in ((0, 256, "A"), (256, 512, "B")):
                        w = hi - lo
                        for p in range(2):
                            o = p * 64
                            rs = nrm.tile([1, w], f32, tag=f"rs{tg}", bufs=2,
                                          name=f"rs{tg}")
                            nc.vector.tensor_copy(rs[:], avs[p][64:65, lo:hi])
                            ri = nrm.tile([1, w], f32, tag=f"ri{tg}", bufs=2,
                                          name=f"ri{tg}")
                            nc.vector.reciprocal_approx_fast(ri[:], rs[:])
                            bc = nrm.tile([64, w], f32, tag=f"bc{tg}", bufs=2,
                                          name=f"bc{tg}")
                            nc.gpsimd.partition_broadcast(bc[:], ri[:])
                            nc.vector.tensor_mul(
                                ctxt[1][o:o + 64, 3 * 512 + lo:3 * 512 + hi],
                                avs[p][0:64, lo:hi],
                                bc[:],
                            )
            while fill_i < len(filler):
                filler[fill_i]()
                fill_i += 1

        # ---- final out-projection rc12-15, two-phase with one PSUM tile per
        # chunk: the u=0 halves run on PSUM freed by the last exp ("a") / last
        # woven outproj ("b") — independent of the final normalize — keeping
        # the PE busy (at full p-state) while the chunked normalize completes;
        # u=1 halves interleave as their ctxt[1] cols become ready. rc15/rc14
        # reuse the a/b rings once the rc12/rc13 drains have read them.
        def hold_u0(rc, n, tag):
            t = ps.tile([128, 512], f32, tag=tag, bufs=2, name=f"h{rc}_{n}")
            nc.tensor.matmul(
                t[:],
                ctxt[0][:, rc * 128:(rc + 1) * 128],
                wo_sb[:, n * 512: n * 512 + 512],
                start=True,
                stop=False,
            )
            return t

        def fin_u1(rc, n, t, eng):
            nc.tensor.matmul(
                t[:],
                ctxt[1][:, rc * 128:(rc + 1) * 128],
                wo_sb[:, DIN + n * 512: DIN + n * 512 + 512],
                start=False,
                stop=True,
            )
            drain_out(rc, n, t[:], eng)

        h12 = [hold_u0(12, n, "a") for n in range(2)]
        h13 = [hold_u0(13, n, "b") for n in range(2)]
        for n in range(2):
            fin_u1(12, n, h12[n], 1)
        for n in range(2):
            fin_u1(13, n, h13[n], 1)
        h15 = [hold_u0(15, n, "a") for n in range(2)]
        h14 = [hold_u0(14, n, "b") for n in range(2)]
        for n in range(2):
            fin_u1(15, n, h15[n], 0)
        for n in range(2):
            fin_u1(14, n, h14[n], 0)

    nc.compile()
    return nc


_NC = None


def _get_nc():
    global _NC
    if _NC is None:
        _NC = build_nc()
    return _NC


def kernel(x, W_q, W_k, W_v, W_o, b_o):
    global LAST_RESULTS
    nc = _get_nc()
    bf = ml_dtypes.bfloat16
    x = np.asarray(x, np.float32).astype(bf)
    # pre-transpose per batch (shared by the 4 cores of each batch)
    xT = [np.ascontiguousarray(x[bi].T) for bi in range(2)]
    W_q = np.asarray(W_q, np.float32).astype(bf)
    W_k = np.asarray(W_k, np.float32).astype(bf)
    W_v = np.asarray(W_v, np.float32).astype(bf)
    W_o = np.asarray(W_o, np.float32).astype(bf)
    b_o = np.asarray(b_o, np.float32).reshape(1, DIN)

    def lay_w(w, sl):   # [1024, 256] shard -> [128, 8*256]: t[p, c*256+n] = w[c*128+p, sl][n]
        return np.ascontiguousarray(
            w[:, sl].reshape(8, 128, QC).transpose(1, 0, 2).reshape(128, 8 * QC))

    def lay_wo(w, sl):  # [256, 1024] shard -> [128, 2*1024]
        return np.ascontiguousarray(
            w[sl, :].reshape(2, 128, DIN).transpose(1, 0, 2).reshape(128, 2 * DIN))

    in_maps = []
    for c in range(8):
        bi, g = c // 4, c % 4
        sl = slice(g * QC, (g + 1) * QC)
        in_maps.append({
            "x": xT[bi],
            "wq": lay_w(W_q, sl),
            "wk": lay_w(W_k, sl),
            "wv": lay_w(W_v, sl),
            "wo": lay_wo(W_o, sl),
        })

    res = run_bass_kernel_spmd(nc, in_maps, list(range(8)), trace=TRACE)
    LAST_RESULTS = res
    outs = [np.asarray(r["out"], dtype=np.float32) for r in res.results]
    return np.stack([
        outs[0] + outs[1] + outs[2] + outs[3] + b_o,
        outs[4] + outs[5] + outs[6] + outs[7] + b_o,
    ])


if __name__ == "__main__":
    if "--compile-only" in sys.argv:
        import tempfile
        from concourse.bass_utils import compile_bass_kernel

        nc = build_nc()
        with tempfile.TemporaryDirectory() as td:
            print("walrus compiling...")
            neff = compile_bass_kernel(nc, td)
            print("COMPILE OK", neff)
